# revision 28
# baseline (speedup 1.0000x reference)
"""DispMVS depth-fusion kernel for 8 Trainium2 NeuronCores (v5).

Sharding: core c = (b, rh, wh): batch b = c // 4, row-half rh = (c // 2) % 2
(128 of 256 coarse rows), col-half wh = c % 2 (160 of 320 coarse cols).
Partitions = 128 rows; BOTH neighbor streams (NN=2) live on the free axis, so
the cross-neighbor confidence fusion is partition-local with full 128-lane
ops and no SBUF->SBUF partition moves.

Per-core pipeline (identical Bass/Tile program on all 8 cores):
  1. geometry: epipolar math -> clipped inverse depth inv[p, nn, 166]
     (host pre-bakes the pixel-only linear fields a_j, b_j, r_j; per-nn
     scalars enter as stride-0 broadcast tensors).  DRAM scratch round-trip
     rebuilds inv as 3 vertically shifted rows t3i [128, 3, 164] per nn.
  2. 8 chunks (nn, qy) of mask [128, qx4, w160, k9] fp16: exp in-place on
     ACT; softmax denominator via grouped tensor_reduce (fp16 in, f32 out);
     numerators via f32 MUL_CUMSUM scans (fp16 e) against interleaved
     unfold-weight tiles, extracting group sums with every-9th strided-diff
     subtracts against a zero-padded prefix.  GpSimd is deliberately idle:
     concurrent GpSimd SBUF traffic degrades DVE throughput ~4x.
  3. confidence fusion batched at the end: sigmoid weights on ACT (one
     table reload), convex combine, affine + reciprocal -> contiguous
     [512, 640] store per core.
"""

import numpy as np

NN, B, H, W = 2, 2, 256, 320
UP = 4
EPS = 1e-6
NCORES = 8
RP = 128          # coarse rows per core
WC = 160          # coarse cols per core
GX = 166          # geometry cols: 162 (= 160 + 2 col-halo) + 4 packed halo-row px
CPAD = 9          # zero prefix cols in cum tile

# broadcast-consts columns (per nn)
C_T0, C_T1, C_T2, C_SA, C_CA, C_SB, C_CB, C_TXN, C_TYN, C_TZ = range(10)
NCB = 10
# per-partition consts columns
K_CA, K_CB, K_DS, K_DB = 0, 1, 2, 3
NCONST = 4

_cache = {}


def _register_custom_ops():
    """Register custom DVE ops (idempotent)."""
    from concourse import dve_ops
    from concourse.dve_spec import AluOp, C0, C1, Spec, Src0, Src1, _has_src1, lower, scan
    from concourse.dve_uop import DveOpSpec

    have = {o.name: o for o in dve_ops.OPS}
    if "MUL_CUMSUM_ANT" in have:
        return have

    def cum_ref(in0, in1, s0, s1, imm2):
        a = in0.astype(np.float32).reshape(in0.shape[0], -1) * in1.astype(
            np.float32
        ).reshape(in1.shape[0], -1)
        return np.cumsum(a, axis=1, dtype=np.float32).reshape(in0.shape)

    specs = [
        ("MUL_CUMSUM_ANT", Spec(body=scan(AluOp.ADD, Src0 * Src1), reference=cum_ref)),
        (
            "SUMSQ_ANT",
            Spec(
                body=Src0 * Src0 + Src1 * Src1,
                reference=lambda in0, in1, s0, s1, imm2: (
                    in0.astype(np.float32) ** 2 + in1.astype(np.float32) ** 2
                ),
            ),
        ),
        (
            "RSQRT_NR_ANT",
            Spec(
                body=(Src0 * Src0 * Src1 * C0 + C1) * Src0,
                reference=lambda in0, in1, s0, s1, imm2: (
                    (in0.astype(np.float32) ** 2 * in1 * s0 + s1) * in0
                ),
            ),
        ),
    ]
    out = dict(have)
    for name, spec in specs:
        op = dve_ops.DveOp(name, spec, subdim=False, uops_sha={})
        dve_ops.OPS.append(op)
        dve_ops.CUSTOM_DVE_SPECS[name] = spec
        dve_ops._SUB_OPCODE_FOR_NAME[name] = (
            dve_ops._CUSTOM_DVE_ROW_BASE + len(dve_ops.OPS) - 1
        )
        for ver in ("v3", "v4"):
            tmp = DveOpSpec(
                name=name,
                opcode=dve_ops.get_dve_sub_opcode(name),
                uops=lower(spec, ver=ver),
                rd1_en=_has_src1(spec),
            )
            op.uops_sha[ver] = tmp.sha(ver)
        out[name] = op
    assert max(dve_ops._SUB_OPCODE_FOR_NAME.values()) < 0x20
    return out


def _build_program():
    import concourse.bass as bass
    import concourse.bacc as bacc
    import concourse.tile as tile
    from concourse import mybir
    from concourse.alu_op_type import AluOpType as op

    f32 = mybir.dt.float32
    bf16 = mybir.dt.bfloat16
    f16 = mybir.dt.float16
    i32 = mybir.dt.int32
    Act = mybir.ActivationFunctionType

    cops = _register_custom_ops()
    nc = bacc.Bacc("TRN2", target_bir_lowering=False, debug=False)

    pix_d = nc.dram_tensor("pix", [128, NN, 2, GX], f32, kind="ExternalInput").ap()
    geo_d = nc.dram_tensor("geo", [128, NN, 9, GX], f32, kind="ExternalInput").ap()
    hm_d = nc.dram_tensor("hm", [128, NN, GX], f32, kind="ExternalInput").ap()
    cbc_d = nc.dram_tensor("cbc", [128, NN, NCB], f32, kind="ExternalInput").ap()
    consts_d = nc.dram_tensor("consts", [128, NCONST], f32, kind="ExternalInput").ap()
    confpad_d = nc.dram_tensor("confpad", [NN, 130, 164], f32, kind="ExternalInput").ap()
    mask_d = nc.dram_tensor("maskpk", [NN, 4, 128, 5760], f16, kind="ExternalInput").ap()
    scr = nc.dram_tensor("scr", [NN, 130, 164], f32, kind="Internal").ap()
    out_d = nc.dram_tensor("out", [RP * UP, WC * UP], f32, kind="ExternalOutput").ap()

    def dram_ap(base, off, dims):
        return bass.AP(tensor=base.tensor, offset=base.offset + off, ap=[list(d) for d in dims])

    def sb_ap(t, off, dims):
        a = t[:]
        return bass.AP(tensor=a.tensor, offset=a.offset + off,
                       ap=[list(a.ap[0])] + [list(d) for d in dims])

    with tile.TileContext(nc) as tc:
        with tc.tile_pool(name="persist", bufs=1) as pp:
            # ---------- persistent loads ----------
            pix = pp.tile([128, NN, 2, GX], f32, name="pix")
            geo = pp.tile([128, NN, 9, GX], f32, name="geo")
            hm = pp.tile([128, NN, GX], f32, name="hm")
            cbc = pp.tile([128, NN, NCB], f32, name="cbc")
            consts = pp.tile([128, NCONST], f32, name="consts")
            nc.sync.dma_start(out=pix[:], in_=pix_d)
            nc.sync.dma_start(out=geo[:], in_=geo_d)
            nc.sync.dma_start(out=hm[:], in_=hm_d)
            nc.sync.dma_start(out=cbc[:], in_=cbc_d)
            nc.sync.dma_start(out=consts[:], in_=consts_d)

            t3c = pp.tile([128, NN, 3, 164], f32, name="t3c")
            for nn in range(NN):
                src = dram_ap(confpad_d, nn * 130 * 164,
                              [[164, 128], [164, 3], [1, 164]])
                nc.sync.dma_start(out=t3c[:, nn], in_=src)

            def CB(nn_, i):
                return cbc[:, nn_, i:i + 1].broadcast_to([128, GX])

            d_ch = pix[:, :, 0, :]     # depth (both nn views identical data per nn)
            fl_ch = pix[:, :, 1, :]    # flow

            ep_ctx = tc.tile_pool(name="geom", bufs=1)
            gp = ep_ctx.__enter__()
            _t = [0]

            def T(shape=(128, NN, GX)):
                _t[0] += 1
                return gp.tile(list(shape), f32, name=f"g{_t[0]}", tag=f"g{_t[0]}")

            def VTT(o, a, b, alu):
                nc.vector.tensor_tensor(out=o, in0=a, in1=b, op=alu)

            def GTT(o, a, b, alu):
                nc.vector.tensor_tensor(out=o, in0=a, in1=b, op=alu)

            def TS(o, a, s1, o0, s2=None, o1=None):
                if o1 is None:
                    nc.vector.tensor_scalar(out=o, in0=a, scalar1=s1, scalar2=None, op0=o0)
                else:
                    nc.vector.tensor_scalar(out=o, in0=a, scalar1=s1, scalar2=s2, op0=o0, op1=o1)

            def STT(o, a, s, b, o0, o1):
                nc.vector.scalar_tensor_tensor(out=o, in0=a, scalar=s, in1=b, op0=o0, op1=o1)

            def AB(o, a):
                nc.scalar.activation(out=o, in_=a, func=Act.Abs)

            # ---------------- geometry ----------------
            # m_j = a_j * d ; ps_j = m_j + t_j ; pe_j = 2*m_j + b_j
            m0, m1, m2 = T(), T(), T()
            ps0, ps1, ps2 = T(), T(), T()
            pe0, pe1, pe2 = T(), T(), T()
            for j, (mj, psj, pej) in enumerate(((m0, ps0, pe0), (m1, ps1, pe1), (m2, ps2, pe2))):
                GTT(mj[:], geo[:, :, j, :], d_ch, op.mult)
                for nn in range(NN):
                    VTT(psj[:, nn], mj[:, nn], CB(nn, C_T0 + j), op.add)
                STT(pej[:], mj[:], 2.0, geo[:, :, 3 + j, :], op.mult, op.add)

            rs2, re2 = T(), T()
            tmp = T()
            rscr = T()
            AB(tmp[:], ps2[:])
            TS(tmp[:], tmp[:], EPS, op.add)
            nc.vector.reciprocal_approx_accurate(out=rs2[:], in_=tmp[:], scratch=rscr[:])
            AB(tmp[:], pe2[:])
            TS(tmp[:], tmp[:], EPS, op.add)
            nc.vector.reciprocal_approx_accurate(out=re2[:], in_=tmp[:], scratch=rscr[:])

            pxs, pys, pxe, pye = T(), T(), T(), T()
            GTT(pxs[:], ps0[:], rs2[:], op.mult)
            GTT(pys[:], ps1[:], rs2[:], op.mult)
            GTT(pxe[:], pe0[:], re2[:], op.mult)
            GTT(pye[:], pe1[:], re2[:], op.mult)

            fdx, fdy = T(), T()
            VTT(fdx[:], pxe[:], pxs[:], op.subtract)
            VTT(fdy[:], pye[:], pys[:], op.subtract)

            # rsqrt(fdx^2+fdy^2): magic seed + 2 Newton steps
            q = T()
            nc.vector._custom_dve(cops["SUMSQ_ANT"], out=q[:], in0=fdx[:], in1=fdy[:])
            y = T()
            yi = y[:].bitcast(i32)
            TS(yi, q[:].bitcast(i32), 1, op.arith_shift_right)
            TS(yi, yi, -1, op.bitwise_xor)
            TS(yi, yi, 0x5F3759DF + 1, op.add)
            y2 = T()
            nc.vector._custom_dve(cops["RSQRT_NR_ANT"], out=y2[:], in0=y[:], in1=q[:], s0=-0.5, s1=1.5)
            nc.vector._custom_dve(cops["RSQRT_NR_ANT"], out=y[:], in0=y2[:], in1=q[:], s0=-0.5, s1=1.5)

            fls = T()
            GTT(fls[:], fl_ch, y[:], op.mult)
            mx, my = T(), T()
            GTT(tmp[:], fdx[:], fls[:], op.mult)
            VTT(mx[:], tmp[:], pxs[:], op.add)
            GTT(tmp[:], fdy[:], fls[:], op.mult)
            VTT(my[:], tmp[:], pys[:], op.add)

            fm = T()
            fmi = fm[:].bitcast(i32)
            ax = T()
            AB(ax[:], fdx[:])
            AB(tmp[:], fdy[:])
            VTT(fmi, tmp[:], ax[:], op.is_gt)

            # n = iK @ [mx, my, 1]: nx = mx*sA + cA ; ny = my*sB + cB  (per nn)
            nx, ny = T(), T()
            for nn in range(NN):
                GTT(tmp[:, nn], mx[:, nn], CB(nn, C_SA), op.mult)
                VTT(nx[:, nn], tmp[:, nn], CB(nn, C_CA), op.add)
                GTT(tmp[:, nn], my[:, nn], CB(nn, C_SB), op.mult)
                VTT(ny[:, nn], tmp[:, nn], CB(nn, C_CB), op.add)

            # select axis first, then one triangulation
            rsel, nsel, tsel = T(), T(), T()
            for nn in range(NN):
                fmn = fm[:, nn].bitcast(i32)
                nc.vector.select(out=rsel[:, nn], mask=fmn, on_true=geo[:, nn, 7, :], on_false=geo[:, nn, 6, :])
                nc.vector.select(out=nsel[:, nn], mask=fmn, on_true=ny[:, nn], on_false=nx[:, nn])
            txb, tyb = T(), T()
            for nn in range(NN):
                nc.scalar.activation(out=txb[:, nn], in_=CB(nn, C_TXN), func=Act.Copy)
                nc.scalar.activation(out=tyb[:, nn], in_=CB(nn, C_TYN), func=Act.Copy)
            for nn in range(NN):
                nc.vector.select(out=tsel[:, nn], mask=fm[:, nn].bitcast(i32), on_true=tyb[:, nn], on_false=txb[:, nn])

            num = T()
            GTT(tmp[:], geo[:, :, 8, :], nsel[:], op.mult)   # rz * nsel
            VTT(tmp[:], rsel[:], tmp[:], op.subtract)
            AB(num[:], tmp[:])
            dn = T()
            for nn in range(NN):
                GTT(dn[:, nn], nsel[:, nn], CB(nn, C_TZ), op.mult)
            VTT(dn[:], dn[:], tsel[:], op.add)               # tz*n + (-t)
            AB(dn[:], dn[:])
            TS(dn[:], dn[:], EPS, op.add)
            rdn = T()
            nc.vector.reciprocal_approx_accurate(out=rdn[:], in_=dn[:], scratch=rscr[:])
            inv = T()
            GTT(inv[:], num[:], rdn[:], op.mult)

            # clip to [0,1] after affine (dmax..dmin normalize), zero invalid px
            invc = T()
            nc.scalar.activation(out=invc[:], in_=inv[:], func=Act.Identity,
                                 scale=consts[:, K_CA:K_CA + 1], bias=consts[:, K_CB:K_CB + 1])
            TS(invc[:], invc[:], 0.0, op.max, 1.0, op.min)
            VTT(invc[:], invc[:], hm[:], op.mult)

            # ------- scratch round-trip -------
            for nn in range(NN):
                base = nn * 130 * 164
                nc.sync.dma_start(
                    out=dram_ap(scr, base + 164, [[164, 128], [1, 162]]),
                    in_=invc[:, nn, 0:162],
                )
                # packed halo rows: cols 162-163 -> scr row 0; 164-165 -> row 129
                nc.sync.dma_start(
                    out=dram_ap(scr, base, [[2, 81], [1, 2]]),
                    in_=invc[0:81, nn, 162:164],
                )
                nc.sync.dma_start(
                    out=dram_ap(scr, base + 129 * 164, [[2, 81], [1, 2]]),
                    in_=invc[0:81, nn, 164:166],
                )
                # zero pad cols 162..163 of rows 0..129 (never read, but keep clean)
            t3i = pp.tile([128, NN, 3, 164], f32, name="t3i")
            for nn in range(NN):
                src = dram_ap(scr, nn * 130 * 164, [[164, 128], [164, 3], [1, 164]])
                nc.sync.dma_start(out=t3i[:, nn], in_=src)

            ep_ctx.__exit__(None, None, None)

            # ---------------- chunk loop ----------------
            iu = pp.tile([128, NN, 4, 640], f32, name="iu")   # per (nn, qy)
            cu = pp.tile([128, NN, 4, 640], f32, name="cu")

            # interleaved unfold weights uf9[p, nn, w, k] (k = dy*3+dx innermost)
            uf9i = pp.tile([128, NN, WC, 9], f32, name="uf9i")
            uf9c = pp.tile([128, NN, WC, 9], f32, name="uf9c")
            for t3, uf9 in ((t3c, uf9c), (t3i, uf9i)):
                for dy in range(3):
                    for dx in range(3):
                        nc.vector.tensor_scalar(
                            out=uf9[:, :, :, dy * 3 + dx],
                            in0=t3[:, :, dy, dx:dx + WC],
                            scalar1=1.0, scalar2=None, op0=op.mult)

            with tc.tile_pool(name="chunk", bufs=2) as cp, tc.tile_pool(
                name="chunk1", bufs=1
            ) as cp1:
                for qy in range(4):
                    for nn in range(NN):
                        e = cp.tile([128, 5760], f16, name="e", tag="e")
                        nc.sync.dma_start(
                            out=e[:],
                            in_=dram_ap(mask_d, (nn * 4 + qy) * 128 * 5760,
                                        [[5760, 128], [1, 5760]]))
                        nc.scalar.activation(out=e[:], in_=e[:], func=Act.Exp)

                        # --- den: grouped reduce over k (fp16 in, f32 out) ---
                        den = cp.tile([128, 640], f32, name="den", tag="den")
                        nc.vector.tensor_reduce(
                            out=den[:], in_=e[:].rearrange("p (g k) -> p g k", k=9),
                            axis=mybir.AxisListType.X, op=op.add)
                        rs = cp.tile([128, 640], f32, name="rs", tag="rs")
                        nc.vector.reciprocal_approx_fast(out=rs[:], in_=den[:])

                        # --- numerators: cumsum + strided diff ---
                        cum = cp.tile([128, 4, CPAD + 1440], f32, name="cum", tag="cum")
                        nc.vector.memset(sb_ap(cum, 0, [[CPAD + 1440, 4], [1, CPAD]]), 0.0)
                        for tag, uf9 in (("c", uf9c), ("i", uf9i)):
                            for qx in range(4):
                                nc.vector._custom_dve(
                                    cops["MUL_CUMSUM_ANT"],
                                    out=sb_ap(cum, qx * (CPAD + 1440) + CPAD, [[1, 1440]]),
                                    in0=sb_ap(e, qx * 1440, [[1, 1440]]),
                                    in1=uf9[:, nn].rearrange("p a b -> p (a b)"))
                            acc = cp.tile([128, 640], f32, name="acc", tag="acc" + tag)
                            nc.vector.tensor_tensor(
                                out=acc[:],
                                in0=sb_ap(cum, CPAD + 8, [[CPAD + 1440, 4], [9, WC]]),
                                in1=sb_ap(cum, CPAD - 1, [[CPAD + 1440, 4], [9, WC]]),
                                op=op.subtract)
                            dst = cu if tag == "c" else iu
                            nc.vector.tensor_tensor(out=dst[:, nn, qy], in0=acc[:], in1=rs[:], op=op.mult)

                # ---------------- fusion (batched) ----------------
                for qy in range(4):
                    dif = cp1.tile([128, 640], f32, name="dif", tag="dif")
                    nc.vector.tensor_tensor(out=dif[:], in0=cu[:, 1, qy], in1=cu[:, 0, qy], op=op.subtract)
                    dm = cp1.tile([128, 640], f32, name="dm", tag="dm")
                    nc.vector.tensor_tensor(out=dm[:], in0=iu[:, 1, qy], in1=iu[:, 0, qy], op=op.subtract)
                    s1 = cp1.tile([128, 640], f32, name="s1", tag="s1")
                    nc.scalar.activation(out=s1[:], in_=dif[:], func=Act.Sigmoid)
                    t = cp1.tile([128, 640], f32, name="t", tag="t")
                    nc.vector.tensor_tensor(out=t[:], in0=s1[:], in1=dm[:], op=op.mult)
                    fus = cp1.tile([128, 640], f32, name="fus", tag="fus")
                    nc.vector.tensor_tensor(out=fus[:], in0=t[:], in1=iu[:, 0, qy], op=op.add)
                    aff = cp1.tile([128, 640], f32, name="aff", tag="aff")
                    nc.scalar.activation(out=aff[:], in_=fus[:], func=Act.Identity,
                                         scale=consts[:, K_DS:K_DS + 1], bias=consts[:, K_DB:K_DB + 1])
                    ot = cp1.tile([128, 160, 4], f32, name="ot", tag="ot")
                    nc.vector.reciprocal_approx_fast(
                        out=ot[:].rearrange("p w q -> p q w"),
                        in_=aff[:].rearrange("p (q w) -> p q w", q=4))
                    nc.sync.dma_start(
                        out=dram_ap(out_d, qy * WC * UP, [[UP * WC * UP, 128], [1, 640]]),
                        in_=ot[:].rearrange("p a b -> p (a b)"))

    nc.finalize()
    return nc


def _host_prep(inputs):
    K_ref = np.asarray(inputs["K_ref"], np.float32)
    K_nei = np.asarray(inputs["K_nei"], np.float32)
    R_nei = np.asarray(inputs["R_nei"], np.float32)
    T_nei = np.asarray(inputs["T_nei"], np.float32)
    depth0 = np.asarray(inputs["depth0"], np.float32)
    flow = np.asarray(inputs["flow"], np.float32)
    mask = np.asarray(inputs["mask"], np.float32)
    conf = np.asarray(inputs["conf"], np.float32)
    dmin = float(np.asarray(inputs["depth_min"]).reshape(-1)[0])
    dmax = float(np.asarray(inputs["depth_max"]).reshape(-1)[0])

    cA = 1.0 / (dmin - dmax)
    cB = -dmax / (dmin - dmax)

    # pixel rays per batch (unit-z), float64 on host
    uv = []
    for b in range(B):
        Ki = np.linalg.inv(K_ref[b, 0, 0].astype(np.float64))
        gx, gy = np.meshgrid(np.arange(W, dtype=np.float64), np.arange(H, dtype=np.float64))
        x = Ki[0, 0] * gx + Ki[0, 1] * gy + Ki[0, 2]
        yy = Ki[1, 0] * gx + Ki[1, 1] * gy + Ki[1, 2]
        z = Ki[2, 0] * gx + Ki[2, 1] * gy + Ki[2, 2]
        uv.append((x / z, yy / z))

    in_maps = []
    for c in range(NCORES):
        b, rh, wh = c // 4, (c // 2) % 2, c % 2
        r0, w0 = rh * RP, wh * WC

        # geometry pixel grid: rows r0..r0+127, cols w0-1..w0+160 (162) +
        # packed halo rows (r0-1, r0+128) x 162 -> cols 162..165
        rows = np.arange(r0, r0 + RP)
        cols = np.clip(np.arange(w0 - 1, w0 + WC + 1), 0, W - 1)  # 162, edge-clamped
        rtop = max(r0 - 1, 0)
        rbot = min(r0 + RP, H - 1)

        pix = np.zeros((128, NN, 2, GX), np.float32)
        geo = np.zeros((128, NN, 9, GX), np.float32)
        hm = np.ones((128, NN, GX), np.float32)
        cbc = np.zeros((128, NN, NCB), np.float32)
        consts = np.zeros((128, NCONST), np.float32)
        consts[:, K_CA] = cA
        consts[:, K_CB] = cB
        consts[:, K_DS] = dmin - dmax
        consts[:, K_DB] = dmax

        ug, vg = uv[b]

        def gather(arr):
            """arr [H, W] -> [128, GX]: main 162 cols + packed halo rows."""
            out = np.zeros((128, GX), np.float32)
            out[:, 0:162] = arr[np.ix_(rows, cols)]
            halo_t = arr[rtop][cols]            # 162
            halo_b = arr[rbot][cols]
            out[0:81, 162:164] = halo_t.reshape(81, 2)
            out[0:81, 164:166] = halo_b.reshape(81, 2)
            return out

        u_g = gather(ug)
        v_g = gather(vg)
        d_g = gather(depth0[b, 0].astype(np.float64))
        for nn in range(NN):
            fl_g = gather(flow[nn, b, 0].astype(np.float64))
            pix[:, nn, 0] = d_g
            pix[:, nn, 1] = fl_g

            Kn = K_nei[nn, b, 0, 0].astype(np.float64)
            Rn = R_nei[nn, b, 0, 0].astype(np.float64)
            Tn = T_nei[nn, b, 0, 0].astype(np.float64).reshape(3)
            M = Kn @ Rn
            t = (Kn @ Tn.reshape(3, 1)).reshape(3)
            iK = np.linalg.inv(Kn)
            assert abs(iK[0, 1]) < 1e-12 and abs(iK[1, 0]) < 1e-12
            assert abs(iK[2, 0]) < 1e-12 and abs(iK[2, 1]) < 1e-12 and abs(iK[2, 2] - 1) < 1e-9

            for j in range(3):
                a_j = M[j, 0] * u_g + M[j, 1] * v_g + M[j, 2]
                geo[:, nn, j] = a_j
                geo[:, nn, 3 + j] = 10.0 * a_j + t[j]
            for j in range(3):
                geo[:, nn, 6 + j] = Rn[j, 0] * u_g + Rn[j, 1] * v_g + Rn[j, 2]

            cbc[:, nn, C_T0] = t[0]
            cbc[:, nn, C_T1] = t[1]
            cbc[:, nn, C_T2] = t[2]
            s = 1.0 + EPS
            cbc[:, nn, C_SA] = iK[0, 0] / s
            cbc[:, nn, C_CA] = iK[0, 2] / s
            cbc[:, nn, C_SB] = iK[1, 1] / s
            cbc[:, nn, C_CB] = iK[1, 2] / s
            cbc[:, nn, C_TXN] = -Tn[0]
            cbc[:, nn, C_TYN] = -Tn[1]
            cbc[:, nn, C_TZ] = Tn[2]

        # hm zeros: invalid halo cols / rows
        if w0 == 0:
            hm[:, :, 0] = 0.0
            hm[0, :, 162] = 0.0   # packed halo rows, left-edge px
            hm[0, :, 164] = 0.0
        if w0 + WC == W:
            hm[:, :, 161] = 0.0
            hm[80, :, 163] = 0.0  # packed halo rows, right-edge px
            hm[80, :, 165] = 0.0
        if r0 == 0:
            hm[:, :, 162:164] = 0.0
        if r0 + RP == H:
            hm[:, :, 164:166] = 0.0
        hm[81:, :, 162:166] = 0.0  # unused packed slots

        confpad = np.zeros((NN, 130, 164), np.float32)
        cw = np.arange(w0 - 1, w0 + WC + 1)
        cwv = (cw >= 0) & (cw < W)
        confpad[:, 1:129, 0:162][:, :, cwv] = conf[:, b, 0, r0:r0 + RP][:, :, cw[cwv]]
        if r0 > 0:
            confpad[:, 0, 0:162][:, cwv] = conf[:, b, 0, r0 - 1][:, cw[cwv]]
        if r0 + RP < H:
            confpad[:, 129, 0:162][:, cwv] = conf[:, b, 0, r0 + RP][:, cw[cwv]]

        # mask: [nn, qy, p, (qx, w, k)] with k = dy*3+dx row-major
        ms = mask[:, b, :, r0:r0 + RP, w0:w0 + WC]          # [NN, 144, 128, 160]
        ms = ms.reshape(NN, 9, 4, 4, RP, WC)               # [NN, k, qy, qx, p, w]
        mask_pk = np.ascontiguousarray(
            ms.transpose(0, 2, 4, 3, 5, 1)                 # [NN, qy, p, qx, w, k]
        ).reshape(NN, 4, 128, 5760).astype(np.float16)

        in_maps.append({
            "pix": pix, "geo": geo, "hm": hm, "cbc": cbc, "consts": consts,
            "confpad": confpad, "maskpk": mask_pk,
        })
    return in_maps


def kernel(**inputs):
    if "nc" not in _cache:
        _cache["nc"] = _build_program()
    nc = _cache["nc"]
    in_maps = _host_prep(inputs)

    from concourse import bass_utils

    res = bass_utils.run_bass_kernel_spmd(nc, in_maps, core_ids=list(range(NCORES)))
    out = np.empty((B, 1, H * UP, W * UP), np.float32)
    for c in range(NCORES):
        b, rh, wh = c // 4, (c // 2) % 2, c % 2
        out[b, 0, rh * RP * UP:(rh + 1) * RP * UP, wh * WC * UP:(wh + 1) * WC * UP] = res.results[c]["out"]
    return out


# revision 29
# speedup vs baseline: 1.0073x; 1.0073x over previous
"""DispMVS depth-fusion kernel for 8 Trainium2 NeuronCores (v5).

Sharding: core c = (b, rh, wh): batch b = c // 4, row-half rh = (c // 2) % 2
(128 of 256 coarse rows), col-half wh = c % 2 (160 of 320 coarse cols).
Partitions = 128 rows; BOTH neighbor streams (NN=2) live on the free axis, so
the cross-neighbor confidence fusion is partition-local with full 128-lane
ops and no SBUF->SBUF partition moves.

Per-core pipeline (identical Bass/Tile program on all 8 cores):
  1. geometry: epipolar math -> clipped inverse depth inv[p, nn, 166]
     (host pre-bakes the pixel-only linear fields a_j, b_j, r_j; per-nn
     scalars enter as stride-0 broadcast tensors).  DRAM scratch round-trip
     rebuilds inv as 3 vertically shifted rows t3i [128, 3, 164] per nn.
  2. 8 chunks (nn, qy) of mask [128, qx4, w160, k9] fp16: exp in-place on
     ACT; softmax denominator via grouped tensor_reduce (fp16 in, f32 out);
     numerators via f32 MUL_CUMSUM scans (fp16 e) against interleaved
     unfold-weight tiles, extracting group sums with every-9th strided-diff
     subtracts against a zero-padded prefix.  GpSimd is deliberately idle:
     concurrent GpSimd SBUF traffic degrades DVE throughput ~4x.
  3. confidence fusion batched at the end: sigmoid weights on ACT (one
     table reload), convex combine, affine + reciprocal -> contiguous
     [512, 640] store per core.
"""

import numpy as np

NN, B, H, W = 2, 2, 256, 320
UP = 4
EPS = 1e-6
NCORES = 8
RP = 128          # coarse rows per core
WC = 160          # coarse cols per core
GX = 166          # geometry cols: 162 (= 160 + 2 col-halo) + 4 packed halo-row px
CPAD = 9          # zero prefix cols in cum tile

# broadcast-consts columns (per nn)
C_T0, C_T1, C_T2, C_SA, C_CA, C_SB, C_CB, C_TXN, C_TYN, C_TZ = range(10)
NCB = 10
# per-partition consts columns
K_CA, K_CB, K_DS, K_DB = 0, 1, 2, 3
NCONST = 4

_cache = {}


def _register_custom_ops():
    """Register custom DVE ops (idempotent)."""
    from concourse import dve_ops
    from concourse.dve_spec import AluOp, C0, C1, Spec, Src0, Src1, _has_src1, lower, scan
    from concourse.dve_uop import DveOpSpec

    have = {o.name: o for o in dve_ops.OPS}
    if "MUL_CUMSUM_ANT" in have:
        return have

    def cum_ref(in0, in1, s0, s1, imm2):
        a = in0.astype(np.float32).reshape(in0.shape[0], -1) * in1.astype(
            np.float32
        ).reshape(in1.shape[0], -1)
        return np.cumsum(a, axis=1, dtype=np.float32).reshape(in0.shape)

    specs = [
        ("MUL_CUMSUM_ANT", Spec(body=scan(AluOp.ADD, Src0 * Src1), reference=cum_ref)),
        (
            "SUMSQ_ANT",
            Spec(
                body=Src0 * Src0 + Src1 * Src1,
                reference=lambda in0, in1, s0, s1, imm2: (
                    in0.astype(np.float32) ** 2 + in1.astype(np.float32) ** 2
                ),
            ),
        ),
        (
            "RSQRT_NR_ANT",
            Spec(
                body=(Src0 * Src0 * Src1 * C0 + C1) * Src0,
                reference=lambda in0, in1, s0, s1, imm2: (
                    (in0.astype(np.float32) ** 2 * in1 * s0 + s1) * in0
                ),
            ),
        ),
    ]
    out = dict(have)
    for name, spec in specs:
        op = dve_ops.DveOp(name, spec, subdim=False, uops_sha={})
        dve_ops.OPS.append(op)
        dve_ops.CUSTOM_DVE_SPECS[name] = spec
        dve_ops._SUB_OPCODE_FOR_NAME[name] = (
            dve_ops._CUSTOM_DVE_ROW_BASE + len(dve_ops.OPS) - 1
        )
        for ver in ("v3", "v4"):
            tmp = DveOpSpec(
                name=name,
                opcode=dve_ops.get_dve_sub_opcode(name),
                uops=lower(spec, ver=ver),
                rd1_en=_has_src1(spec),
            )
            op.uops_sha[ver] = tmp.sha(ver)
        out[name] = op
    assert max(dve_ops._SUB_OPCODE_FOR_NAME.values()) < 0x20
    return out


def _build_program():
    import concourse.bass as bass
    import concourse.bacc as bacc
    import concourse.tile as tile
    from concourse import mybir
    from concourse.alu_op_type import AluOpType as op

    f32 = mybir.dt.float32
    bf16 = mybir.dt.bfloat16
    f16 = mybir.dt.float16
    i32 = mybir.dt.int32
    Act = mybir.ActivationFunctionType

    cops = _register_custom_ops()
    nc = bacc.Bacc("TRN2", target_bir_lowering=False, debug=False)

    pix_d = nc.dram_tensor("pix", [128, NN, 2, GX], f32, kind="ExternalInput").ap()
    geo_d = nc.dram_tensor("geo", [128, NN, 9, GX], f32, kind="ExternalInput").ap()
    hm_d = nc.dram_tensor("hm", [128, NN, GX], f32, kind="ExternalInput").ap()
    cbc_d = nc.dram_tensor("cbc", [128, NN, NCB], f32, kind="ExternalInput").ap()
    consts_d = nc.dram_tensor("consts", [128, NCONST], f32, kind="ExternalInput").ap()
    confpad_d = nc.dram_tensor("confpad", [NN, 130, 164], f32, kind="ExternalInput").ap()
    mask_d = nc.dram_tensor("maskpk", [NN, 4, 128, 5760], f16, kind="ExternalInput").ap()
    scr = nc.dram_tensor("scr", [NN, 130, 164], f32, kind="Internal").ap()
    out_d = nc.dram_tensor("out", [RP * UP, WC * UP], f32, kind="ExternalOutput").ap()

    def dram_ap(base, off, dims):
        return bass.AP(tensor=base.tensor, offset=base.offset + off, ap=[list(d) for d in dims])

    def sb_ap(t, off, dims):
        a = t[:]
        return bass.AP(tensor=a.tensor, offset=a.offset + off,
                       ap=[list(a.ap[0])] + [list(d) for d in dims])

    with tile.TileContext(nc) as tc:
        with tc.tile_pool(name="persist", bufs=1) as pp:
            # ---------- persistent loads ----------
            pix = pp.tile([128, NN, 2, GX], f32, name="pix")
            geo = pp.tile([128, NN, 9, GX], f32, name="geo")
            hm = pp.tile([128, NN, GX], f32, name="hm")
            cbc = pp.tile([128, NN, NCB], f32, name="cbc")
            consts = pp.tile([128, NCONST], f32, name="consts")
            nc.sync.dma_start(out=pix[:], in_=pix_d)
            nc.sync.dma_start(out=geo[:], in_=geo_d)
            nc.sync.dma_start(out=hm[:], in_=hm_d)
            nc.sync.dma_start(out=cbc[:], in_=cbc_d)
            nc.sync.dma_start(out=consts[:], in_=consts_d)

            t3c = pp.tile([128, NN, 3, 164], f32, name="t3c")
            for nn in range(NN):
                src = dram_ap(confpad_d, nn * 130 * 164,
                              [[164, 128], [164, 3], [1, 164]])
                nc.sync.dma_start(out=t3c[:, nn], in_=src)

            def CB(nn_, i):
                return cbc[:, nn_, i:i + 1].broadcast_to([128, GX])

            d_ch = pix[:, :, 0, :]     # depth (both nn views identical data per nn)
            fl_ch = pix[:, :, 1, :]    # flow

            ep_ctx = tc.tile_pool(name="geom", bufs=1)
            gp = ep_ctx.__enter__()
            _t = [0]

            def T(shape=(128, NN, GX)):
                _t[0] += 1
                return gp.tile(list(shape), f32, name=f"g{_t[0]}", tag=f"g{_t[0]}")

            def VTT(o, a, b, alu):
                nc.vector.tensor_tensor(out=o, in0=a, in1=b, op=alu)

            def GTT(o, a, b, alu):
                nc.vector.tensor_tensor(out=o, in0=a, in1=b, op=alu)

            def TS(o, a, s1, o0, s2=None, o1=None):
                if o1 is None:
                    nc.vector.tensor_scalar(out=o, in0=a, scalar1=s1, scalar2=None, op0=o0)
                else:
                    nc.vector.tensor_scalar(out=o, in0=a, scalar1=s1, scalar2=s2, op0=o0, op1=o1)

            def STT(o, a, s, b, o0, o1):
                nc.vector.scalar_tensor_tensor(out=o, in0=a, scalar=s, in1=b, op0=o0, op1=o1)

            def AB(o, a):
                nc.scalar.activation(out=o, in_=a, func=Act.Abs)

            # ---------------- geometry ----------------
            # m_j = a_j * d ; ps_j = m_j + t_j ; pe_j = 2*m_j + b_j
            m0, m1, m2 = T(), T(), T()
            ps0, ps1, ps2 = T(), T(), T()
            pe0, pe1, pe2 = T(), T(), T()
            for j, (mj, psj, pej) in enumerate(((m0, ps0, pe0), (m1, ps1, pe1), (m2, ps2, pe2))):
                GTT(mj[:], geo[:, :, j, :], d_ch, op.mult)
                for nn in range(NN):
                    VTT(psj[:, nn], mj[:, nn], CB(nn, C_T0 + j), op.add)
                STT(pej[:], mj[:], 2.0, geo[:, :, 3 + j, :], op.mult, op.add)

            rs2, re2 = T(), T()
            tmp = T()
            rscr = T()
            AB(tmp[:], ps2[:])
            TS(tmp[:], tmp[:], EPS, op.add)
            nc.vector.reciprocal_approx_accurate(out=rs2[:], in_=tmp[:], scratch=rscr[:])
            AB(tmp[:], pe2[:])
            TS(tmp[:], tmp[:], EPS, op.add)
            nc.vector.reciprocal_approx_accurate(out=re2[:], in_=tmp[:], scratch=rscr[:])

            pxs, pys, pxe, pye = T(), T(), T(), T()
            GTT(pxs[:], ps0[:], rs2[:], op.mult)
            GTT(pys[:], ps1[:], rs2[:], op.mult)
            GTT(pxe[:], pe0[:], re2[:], op.mult)
            GTT(pye[:], pe1[:], re2[:], op.mult)

            fdx, fdy = T(), T()
            VTT(fdx[:], pxe[:], pxs[:], op.subtract)
            VTT(fdy[:], pye[:], pys[:], op.subtract)

            # rsqrt(fdx^2+fdy^2): magic seed + 2 Newton steps
            q = T()
            nc.vector._custom_dve(cops["SUMSQ_ANT"], out=q[:], in0=fdx[:], in1=fdy[:])
            y = T()
            yi = y[:].bitcast(i32)
            TS(yi, q[:].bitcast(i32), 1, op.arith_shift_right)
            TS(yi, yi, -1, op.bitwise_xor)
            TS(yi, yi, 0x5F3759DF + 1, op.add)
            y2 = T()
            nc.vector._custom_dve(cops["RSQRT_NR_ANT"], out=y2[:], in0=y[:], in1=q[:], s0=-0.5, s1=1.5)
            nc.vector._custom_dve(cops["RSQRT_NR_ANT"], out=y[:], in0=y2[:], in1=q[:], s0=-0.5, s1=1.5)

            fls = T()
            GTT(fls[:], fl_ch, y[:], op.mult)
            mx, my = T(), T()
            GTT(tmp[:], fdx[:], fls[:], op.mult)
            VTT(mx[:], tmp[:], pxs[:], op.add)
            GTT(tmp[:], fdy[:], fls[:], op.mult)
            VTT(my[:], tmp[:], pys[:], op.add)

            fm = T()
            fmi = fm[:].bitcast(i32)
            ax = T()
            AB(ax[:], fdx[:])
            AB(tmp[:], fdy[:])
            VTT(fmi, tmp[:], ax[:], op.is_gt)

            # n = iK @ [mx, my, 1]: nx = mx*sA + cA ; ny = my*sB + cB  (per nn)
            nx, ny = T(), T()
            for nn in range(NN):
                GTT(tmp[:, nn], mx[:, nn], CB(nn, C_SA), op.mult)
                VTT(nx[:, nn], tmp[:, nn], CB(nn, C_CA), op.add)
                GTT(tmp[:, nn], my[:, nn], CB(nn, C_SB), op.mult)
                VTT(ny[:, nn], tmp[:, nn], CB(nn, C_CB), op.add)

            # select axis first, then one triangulation
            rsel, nsel, tsel = T(), T(), T()
            for nn in range(NN):
                fmn = fm[:, nn].bitcast(i32)
                nc.vector.select(out=rsel[:, nn], mask=fmn, on_true=geo[:, nn, 7, :], on_false=geo[:, nn, 6, :])
                nc.vector.select(out=nsel[:, nn], mask=fmn, on_true=ny[:, nn], on_false=nx[:, nn])
            txb, tyb = T(), T()
            for nn in range(NN):
                nc.scalar.activation(out=txb[:, nn], in_=CB(nn, C_TXN), func=Act.Copy)
                nc.scalar.activation(out=tyb[:, nn], in_=CB(nn, C_TYN), func=Act.Copy)
            for nn in range(NN):
                nc.vector.select(out=tsel[:, nn], mask=fm[:, nn].bitcast(i32), on_true=tyb[:, nn], on_false=txb[:, nn])

            num = T()
            GTT(tmp[:], geo[:, :, 8, :], nsel[:], op.mult)   # rz * nsel
            VTT(tmp[:], rsel[:], tmp[:], op.subtract)
            AB(num[:], tmp[:])
            dn = T()
            for nn in range(NN):
                GTT(dn[:, nn], nsel[:, nn], CB(nn, C_TZ), op.mult)
            VTT(dn[:], dn[:], tsel[:], op.add)               # tz*n + (-t)
            AB(dn[:], dn[:])
            TS(dn[:], dn[:], EPS, op.add)
            rdn = T()
            nc.vector.reciprocal_approx_accurate(out=rdn[:], in_=dn[:], scratch=rscr[:])
            inv = T()
            GTT(inv[:], num[:], rdn[:], op.mult)

            # clip to [0,1] after affine (dmax..dmin normalize), zero invalid px
            invc = T()
            nc.scalar.activation(out=invc[:], in_=inv[:], func=Act.Identity,
                                 scale=consts[:, K_CA:K_CA + 1], bias=consts[:, K_CB:K_CB + 1])
            TS(invc[:], invc[:], 0.0, op.max, 1.0, op.min)
            VTT(invc[:], invc[:], hm[:], op.mult)

            # ------- scratch round-trip -------
            for nn in range(NN):
                base = nn * 130 * 164
                nc.sync.dma_start(
                    out=dram_ap(scr, base + 164, [[164, 128], [1, 162]]),
                    in_=invc[:, nn, 0:162],
                )
                # packed halo rows: cols 162-163 -> scr row 0; 164-165 -> row 129
                nc.sync.dma_start(
                    out=dram_ap(scr, base, [[2, 81], [1, 2]]),
                    in_=invc[0:81, nn, 162:164],
                )
                nc.sync.dma_start(
                    out=dram_ap(scr, base + 129 * 164, [[2, 81], [1, 2]]),
                    in_=invc[0:81, nn, 164:166],
                )
                # zero pad cols 162..163 of rows 0..129 (never read, but keep clean)
            t3i = pp.tile([128, NN, 3, 164], f32, name="t3i")
            for nn in range(NN):
                src = dram_ap(scr, nn * 130 * 164, [[164, 128], [164, 3], [1, 164]])
                nc.sync.dma_start(out=t3i[:, nn], in_=src)

            ep_ctx.__exit__(None, None, None)

            # ---------------- chunk loop ----------------
            iu = pp.tile([128, NN, 4, 640], f32, name="iu")   # per (nn, qy)
            cu = pp.tile([128, NN, 4, 640], f32, name="cu")

            # interleaved unfold weights uf9[p, nn, w, k] (k = dy*3+dx innermost)
            uf9i = pp.tile([128, NN, WC, 9], f32, name="uf9i")
            uf9c = pp.tile([128, NN, WC, 9], f32, name="uf9c")
            for t3, uf9 in ((t3c, uf9c), (t3i, uf9i)):
                for dy in range(3):
                    for dx in range(3):
                        nc.vector.tensor_scalar(
                            out=uf9[:, :, :, dy * 3 + dx],
                            in0=t3[:, :, dy, dx:dx + WC],
                            scalar1=1.0, scalar2=None, op0=op.mult)

            with tc.tile_pool(name="chunk", bufs=2) as cp, tc.tile_pool(
                name="chunk1", bufs=1
            ) as cp1:
                for qy in range(4):
                    for nn in range(NN):
                        e = cp.tile([128, 5760], f16, name="e", tag="e")
                        nc.sync.dma_start(
                            out=e[:],
                            in_=dram_ap(mask_d, (nn * 4 + qy) * 128 * 5760,
                                        [[5760, 128], [1, 5760]]))
                        nc.scalar.activation(out=e[:], in_=e[:], func=Act.Exp)

                        # --- den: grouped reduce over k (fp16 in, f32 out) ---
                        den = cp.tile([128, 640], f32, name="den", tag="den")
                        nc.vector.tensor_reduce(
                            out=den[:], in_=e[:].rearrange("p (g k) -> p g k", k=9),
                            axis=mybir.AxisListType.X, op=op.add)
                        rs = cp.tile([128, 640], f32, name="rs", tag="rs")
                        nc.vector.reciprocal_approx_fast(out=rs[:], in_=den[:])

                        # --- numerators: cumsum + strided diff ---
                        cum = cp.tile([128, CPAD + 5760], f32, name="cum", tag="cum")
                        nc.vector.memset(sb_ap(cum, 0, [[1, CPAD]]), 0.0)
                        for tag, uf9 in (("c", uf9c), ("i", uf9i)):
                            u = uf9[:, nn]
                            uap = bass.AP(tensor=u.tensor, offset=u.offset,
                                          ap=[list(u.ap[0]), [0, 4], [1, 1440]])
                            nc.vector._custom_dve(
                                cops["MUL_CUMSUM_ANT"],
                                out=sb_ap(cum, CPAD, [[1, 5760]]),
                                in0=sb_ap(e, 0, [[1, 5760]]),
                                in1=uap)
                            acc = cp.tile([128, 640], f32, name="acc", tag="acc" + tag)
                            nc.vector.tensor_tensor(
                                out=acc[:],
                                in0=sb_ap(cum, CPAD + 8, [[9, 640]]),
                                in1=sb_ap(cum, CPAD - 1, [[9, 640]]),
                                op=op.subtract)
                            dst = cu if tag == "c" else iu
                            nc.vector.tensor_tensor(out=dst[:, nn, qy], in0=acc[:], in1=rs[:], op=op.mult)

                # ---------------- fusion (batched) ----------------
                for qy in range(4):
                    dif = cp1.tile([128, 640], f32, name="dif", tag="dif")
                    nc.vector.tensor_tensor(out=dif[:], in0=cu[:, 1, qy], in1=cu[:, 0, qy], op=op.subtract)
                    dm = cp1.tile([128, 640], f32, name="dm", tag="dm")
                    nc.vector.tensor_tensor(out=dm[:], in0=iu[:, 1, qy], in1=iu[:, 0, qy], op=op.subtract)
                    s1 = cp1.tile([128, 640], f32, name="s1", tag="s1")
                    nc.scalar.activation(out=s1[:], in_=dif[:], func=Act.Sigmoid)
                    t = cp1.tile([128, 640], f32, name="t", tag="t")
                    nc.vector.tensor_tensor(out=t[:], in0=s1[:], in1=dm[:], op=op.mult)
                    fus = cp1.tile([128, 640], f32, name="fus", tag="fus")
                    nc.vector.tensor_tensor(out=fus[:], in0=t[:], in1=iu[:, 0, qy], op=op.add)
                    aff = cp1.tile([128, 640], f32, name="aff", tag="aff")
                    nc.scalar.activation(out=aff[:], in_=fus[:], func=Act.Identity,
                                         scale=consts[:, K_DS:K_DS + 1], bias=consts[:, K_DB:K_DB + 1])
                    ot = cp1.tile([128, 160, 4], f32, name="ot", tag="ot")
                    nc.vector.reciprocal_approx_fast(
                        out=ot[:].rearrange("p w q -> p q w"),
                        in_=aff[:].rearrange("p (q w) -> p q w", q=4))
                    nc.sync.dma_start(
                        out=dram_ap(out_d, qy * WC * UP, [[UP * WC * UP, 128], [1, 640]]),
                        in_=ot[:].rearrange("p a b -> p (a b)"))

    nc.finalize()
    return nc


def _host_prep(inputs):
    K_ref = np.asarray(inputs["K_ref"], np.float32)
    K_nei = np.asarray(inputs["K_nei"], np.float32)
    R_nei = np.asarray(inputs["R_nei"], np.float32)
    T_nei = np.asarray(inputs["T_nei"], np.float32)
    depth0 = np.asarray(inputs["depth0"], np.float32)
    flow = np.asarray(inputs["flow"], np.float32)
    mask = np.asarray(inputs["mask"], np.float32)
    conf = np.asarray(inputs["conf"], np.float32)
    dmin = float(np.asarray(inputs["depth_min"]).reshape(-1)[0])
    dmax = float(np.asarray(inputs["depth_max"]).reshape(-1)[0])

    cA = 1.0 / (dmin - dmax)
    cB = -dmax / (dmin - dmax)

    # pixel rays per batch (unit-z), float64 on host
    uv = []
    for b in range(B):
        Ki = np.linalg.inv(K_ref[b, 0, 0].astype(np.float64))
        gx, gy = np.meshgrid(np.arange(W, dtype=np.float64), np.arange(H, dtype=np.float64))
        x = Ki[0, 0] * gx + Ki[0, 1] * gy + Ki[0, 2]
        yy = Ki[1, 0] * gx + Ki[1, 1] * gy + Ki[1, 2]
        z = Ki[2, 0] * gx + Ki[2, 1] * gy + Ki[2, 2]
        uv.append((x / z, yy / z))

    in_maps = []
    for c in range(NCORES):
        b, rh, wh = c // 4, (c // 2) % 2, c % 2
        r0, w0 = rh * RP, wh * WC

        # geometry pixel grid: rows r0..r0+127, cols w0-1..w0+160 (162) +
        # packed halo rows (r0-1, r0+128) x 162 -> cols 162..165
        rows = np.arange(r0, r0 + RP)
        cols = np.clip(np.arange(w0 - 1, w0 + WC + 1), 0, W - 1)  # 162, edge-clamped
        rtop = max(r0 - 1, 0)
        rbot = min(r0 + RP, H - 1)

        pix = np.zeros((128, NN, 2, GX), np.float32)
        geo = np.zeros((128, NN, 9, GX), np.float32)
        hm = np.ones((128, NN, GX), np.float32)
        cbc = np.zeros((128, NN, NCB), np.float32)
        consts = np.zeros((128, NCONST), np.float32)
        consts[:, K_CA] = cA
        consts[:, K_CB] = cB
        consts[:, K_DS] = dmin - dmax
        consts[:, K_DB] = dmax

        ug, vg = uv[b]

        def gather(arr):
            """arr [H, W] -> [128, GX]: main 162 cols + packed halo rows."""
            out = np.zeros((128, GX), np.float32)
            out[:, 0:162] = arr[np.ix_(rows, cols)]
            halo_t = arr[rtop][cols]            # 162
            halo_b = arr[rbot][cols]
            out[0:81, 162:164] = halo_t.reshape(81, 2)
            out[0:81, 164:166] = halo_b.reshape(81, 2)
            return out

        u_g = gather(ug)
        v_g = gather(vg)
        d_g = gather(depth0[b, 0].astype(np.float64))
        for nn in range(NN):
            fl_g = gather(flow[nn, b, 0].astype(np.float64))
            pix[:, nn, 0] = d_g
            pix[:, nn, 1] = fl_g

            Kn = K_nei[nn, b, 0, 0].astype(np.float64)
            Rn = R_nei[nn, b, 0, 0].astype(np.float64)
            Tn = T_nei[nn, b, 0, 0].astype(np.float64).reshape(3)
            M = Kn @ Rn
            t = (Kn @ Tn.reshape(3, 1)).reshape(3)
            iK = np.linalg.inv(Kn)
            assert abs(iK[0, 1]) < 1e-12 and abs(iK[1, 0]) < 1e-12
            assert abs(iK[2, 0]) < 1e-12 and abs(iK[2, 1]) < 1e-12 and abs(iK[2, 2] - 1) < 1e-9

            for j in range(3):
                a_j = M[j, 0] * u_g + M[j, 1] * v_g + M[j, 2]
                geo[:, nn, j] = a_j
                geo[:, nn, 3 + j] = 10.0 * a_j + t[j]
            for j in range(3):
                geo[:, nn, 6 + j] = Rn[j, 0] * u_g + Rn[j, 1] * v_g + Rn[j, 2]

            cbc[:, nn, C_T0] = t[0]
            cbc[:, nn, C_T1] = t[1]
            cbc[:, nn, C_T2] = t[2]
            s = 1.0 + EPS
            cbc[:, nn, C_SA] = iK[0, 0] / s
            cbc[:, nn, C_CA] = iK[0, 2] / s
            cbc[:, nn, C_SB] = iK[1, 1] / s
            cbc[:, nn, C_CB] = iK[1, 2] / s
            cbc[:, nn, C_TXN] = -Tn[0]
            cbc[:, nn, C_TYN] = -Tn[1]
            cbc[:, nn, C_TZ] = Tn[2]

        # hm zeros: invalid halo cols / rows
        if w0 == 0:
            hm[:, :, 0] = 0.0
            hm[0, :, 162] = 0.0   # packed halo rows, left-edge px
            hm[0, :, 164] = 0.0
        if w0 + WC == W:
            hm[:, :, 161] = 0.0
            hm[80, :, 163] = 0.0  # packed halo rows, right-edge px
            hm[80, :, 165] = 0.0
        if r0 == 0:
            hm[:, :, 162:164] = 0.0
        if r0 + RP == H:
            hm[:, :, 164:166] = 0.0
        hm[81:, :, 162:166] = 0.0  # unused packed slots

        confpad = np.zeros((NN, 130, 164), np.float32)
        cw = np.arange(w0 - 1, w0 + WC + 1)
        cwv = (cw >= 0) & (cw < W)
        confpad[:, 1:129, 0:162][:, :, cwv] = conf[:, b, 0, r0:r0 + RP][:, :, cw[cwv]]
        if r0 > 0:
            confpad[:, 0, 0:162][:, cwv] = conf[:, b, 0, r0 - 1][:, cw[cwv]]
        if r0 + RP < H:
            confpad[:, 129, 0:162][:, cwv] = conf[:, b, 0, r0 + RP][:, cw[cwv]]

        # mask: [nn, qy, p, (qx, w, k)] with k = dy*3+dx row-major
        ms = mask[:, b, :, r0:r0 + RP, w0:w0 + WC]          # [NN, 144, 128, 160]
        ms = ms.reshape(NN, 9, 4, 4, RP, WC)               # [NN, k, qy, qx, p, w]
        mask_pk = np.ascontiguousarray(
            ms.transpose(0, 2, 4, 3, 5, 1)                 # [NN, qy, p, qx, w, k]
        ).reshape(NN, 4, 128, 5760).astype(np.float16)

        in_maps.append({
            "pix": pix, "geo": geo, "hm": hm, "cbc": cbc, "consts": consts,
            "confpad": confpad, "maskpk": mask_pk,
        })
    return in_maps


def kernel(**inputs):
    if "nc" not in _cache:
        _cache["nc"] = _build_program()
    nc = _cache["nc"]
    in_maps = _host_prep(inputs)

    from concourse import bass_utils

    res = bass_utils.run_bass_kernel_spmd(nc, in_maps, core_ids=list(range(NCORES)))
    out = np.empty((B, 1, H * UP, W * UP), np.float32)
    for c in range(NCORES):
        b, rh, wh = c // 4, (c // 2) % 2, c % 2
        out[b, 0, rh * RP * UP:(rh + 1) * RP * UP, wh * WC * UP:(wh + 1) * WC * UP] = res.results[c]["out"]
    return out
